# revision 1
# baseline (speedup 1.0000x reference)
"""Trainium2 Bass kernel for nn_BinaryDense: y = nmk * (x @ tanh(kk*W)) + bias
(soft branch, kk < 1000) or y = nmk * (x @ sign(W)) + bias (hard branch).

Strategy: data-parallel shard of x over its row dim across 8 NeuronCores,
kernel/bias replicated. Per core: [1024, 4096] @ [4096, 4096] in bf16 with
fp32 PSUM accumulation. W is streamed through the scalar engine's tanh LUT
(fp32 -> bf16 cast fused); x is cast (and scaled by nmk) on the vector
engine once and stays SBUF-resident.
"""
import sys

sys.path.insert(0, "/opt/trn_rl_repo")

import numpy as np

N_CORES = 8
P = 128

KK_THRESHOLD = 1000.0

_PROGRAM_CACHE = {}


def _build_program(M, K, N, nmk, kk, use_bias):
    import concourse.bacc as bacc
    import concourse.mybir as mybir
    from concourse.tile import TileContext

    fp32 = mybir.dt.float32
    bf16 = mybir.dt.bfloat16

    KO = K // P          # k-tiles of 128
    MT = M // P          # out-tile rows
    NTILE = 512
    NT = N // NTILE      # out-tile cols

    nc = bacc.Bacc()
    xt = nc.dram_tensor("xt", [K, M], fp32, kind="ExternalInput")
    w = nc.dram_tensor("w", [K, N], fp32, kind="ExternalInput")
    if use_bias:
        bias = nc.dram_tensor("bias", [1, N], fp32, kind="ExternalInput")
    out = nc.dram_tensor("out", [M, N], fp32, kind="ExternalOutput")

    xt_r = xt.rearrange("(ko p) m -> p ko m", p=P)
    w_r = w.rearrange("(ko p) n -> p ko n", p=P)
    out_r = out.rearrange("(mo p) n -> p mo n", p=P)

    wfunc = (
        mybir.ActivationFunctionType.Tanh
        if kk < KK_THRESHOLD
        else mybir.ActivationFunctionType.Sign
    )
    wscale = float(kk) if kk < KK_THRESHOLD else 1.0

    with TileContext(nc) as tc:
        with tc.tile_pool(name="const", bufs=1) as const, \
             tc.tile_pool(name="xstage", bufs=4) as xstage, \
             tc.tile_pool(name="wstage", bufs=6) as wstage, \
             tc.tile_pool(name="wpool", bufs=2) as wpool, \
             tc.tile_pool(name="opool", bufs=4) as opool, \
             tc.tile_pool(name="psum", bufs=8, space="PSUM") as psum:

            # x: load fp32, scale by nmk, cast to bf16; stays resident.
            xt_bf = const.tile([P, KO, M], bf16)
            for ko in range(KO):
                xs = xstage.tile([P, M], fp32, tag="xs")
                nc.sync.dma_start(out=xs, in_=xt_r[:, ko])
                if nmk != 1.0:
                    nc.vector.tensor_scalar_mul(xt_bf[:, ko], xs, float(nmk))
                else:
                    nc.vector.tensor_copy(out=xt_bf[:, ko], in_=xs)

            if use_bias:
                ones_bf = const.tile([1, P], bf16)
                nc.any.memset(ones_bf, 1.0)
                bias_sb = const.tile([1, N], fp32)
                nc.sync.dma_start(out=bias_sb, in_=bias[:])
                bias_bf = const.tile([1, N], bf16)
                nc.vector.tensor_copy(out=bias_bf, in_=bias_sb)

            for nt in range(NT):
                # W n-slice: stream fp32 in, tanh(kk*.) -> bf16 on ScalarE.
                wb = wpool.tile([P, KO, NTILE], bf16, tag="wb")
                for ko in range(KO):
                    ws = wstage.tile([P, NTILE], fp32, tag="ws")
                    nc.sync.dma_start(
                        out=ws, in_=w_r[:, ko, nt * NTILE:(nt + 1) * NTILE]
                    )
                    nc.scalar.activation(
                        out=wb[:, ko], in_=ws, func=wfunc, scale=wscale
                    )
                for mt in range(MT):
                    ps = psum.tile([P, NTILE], fp32)
                    for ko in range(KO):
                        nc.tensor.matmul(
                            ps,
                            xt_bf[:, ko, mt * P:(mt + 1) * P],
                            wb[:, ko],
                            start=(ko == 0),
                            stop=(ko == KO - 1) and not use_bias,
                        )
                    if use_bias:
                        nc.tensor.matmul(
                            ps,
                            ones_bf,
                            bias_bf[:, nt * NTILE:(nt + 1) * NTILE],
                            start=False,
                            stop=True,
                        )
                    ob = opool.tile([P, NTILE], fp32, tag="ob")
                    nc.vector.tensor_copy(out=ob, in_=ps)
                    nc.sync.dma_start(
                        out=out_r[:, mt, nt * NTILE:(nt + 1) * NTILE], in_=ob
                    )

    nc.finalize()
    return nc


def kernel(x, kernel, bias, nmk, kk):
    from concourse.bass_utils import run_bass_kernel_spmd

    x = np.asarray(x)
    w = np.ascontiguousarray(np.asarray(kernel), dtype=np.float32)
    bias = np.asarray(bias, dtype=np.float32)
    nmk_f = float(np.asarray(nmk))
    kk_f = float(np.asarray(kk))

    M_full, K = x.shape
    _, N = w.shape
    assert M_full % N_CORES == 0
    M = M_full // N_CORES

    use_bias = bool(np.any(bias))

    key = (M, K, N, nmk_f, kk_f, use_bias)
    nc = _PROGRAM_CACHE.get(key)
    if nc is None:
        nc = _build_program(M, K, N, nmk_f, kk_f, use_bias)
        _PROGRAM_CACHE[key] = nc

    in_maps = []
    for i in range(N_CORES):
        m = {
            "xt": np.ascontiguousarray(x[i * M:(i + 1) * M, :].T, dtype=np.float32),
            "w": w,
        }
        if use_bias:
            m["bias"] = np.ascontiguousarray(bias.reshape(1, N))
        in_maps.append(m)

    res = run_bass_kernel_spmd(nc, in_maps, core_ids=list(range(N_CORES)))
    out = np.concatenate([r["out"] for r in res.results], axis=0)
    return out.astype(np.float32, copy=False)


# revision 5
# speedup vs baseline: 1.0050x; 1.0050x over previous
"""Trainium2 Bass kernel for nn_BinaryDense: y = nmk * (x @ tanh(kk*W)) + bias
(soft branch, kk < 1000) or y = nmk * (x @ sign(W)) + bias (hard branch).

Strategy: data-parallel shard of x over its row dim across 8 NeuronCores,
kernel/bias replicated. Per core: [1024, 4096] @ [4096, 4096] in bf16 with
fp32 PSUM accumulation. W is streamed through the scalar engine's tanh LUT
(fp32 -> bf16 cast fused); x is cast (and scaled by nmk) on the vector
engine once and stays SBUF-resident.
"""
import sys

sys.path.insert(0, "/opt/trn_rl_repo")

import numpy as np

N_CORES = 8
P = 128

KK_THRESHOLD = 1000.0

_PROGRAM_CACHE = {}


def _build_program(M, K, N, nmk, kk, use_bias):
    import concourse.bacc as bacc
    import concourse.mybir as mybir
    from concourse.tile import TileContext

    fp32 = mybir.dt.float32
    bf16 = mybir.dt.bfloat16

    KO = K // P          # k-tiles of 128
    MT = M // P          # out-tile rows
    NTILE = 512
    NT = N // NTILE      # out-tile cols

    nc = bacc.Bacc()
    xt = nc.dram_tensor("xt", [K, M], fp32, kind="ExternalInput")
    w = nc.dram_tensor("w", [K, N], fp32, kind="ExternalInput")
    if use_bias:
        bias = nc.dram_tensor("bias", [1, N], fp32, kind="ExternalInput")
    out = nc.dram_tensor("out", [M, N], fp32, kind="ExternalOutput")

    xt_r = xt.rearrange("(ko p) m -> p ko m", p=P)
    w_r = w.rearrange("(ko p) n -> p ko n", p=P)
    out_r = out.rearrange("(mo p) n -> p mo n", p=P)

    wfunc = (
        mybir.ActivationFunctionType.Tanh
        if kk < KK_THRESHOLD
        else mybir.ActivationFunctionType.Sign
    )
    wscale = float(kk) if kk < KK_THRESHOLD else 1.0

    with TileContext(nc) as tc:
        with tc.tile_pool(name="const", bufs=1) as const, \
             tc.tile_pool(name="xstage", bufs=4) as xstage, \
             tc.tile_pool(name="wstage", bufs=6) as wstage, \
             tc.tile_pool(name="wpool", bufs=2) as wpool, \
             tc.tile_pool(name="opool", bufs=4) as opool, \
             tc.tile_pool(name="psum", bufs=8, space="PSUM") as psum:

            # x: load fp32, scale by nmk, cast to bf16; stays resident.
            xt_bf = const.tile([P, KO, M], bf16)
            for ko in range(KO):
                xs = xstage.tile([P, M], fp32, tag="xs")
                nc.sync.dma_start(out=xs, in_=xt_r[:, ko])
                if nmk != 1.0:
                    nc.vector.tensor_scalar_mul(xt_bf[:, ko], xs, float(nmk))
                else:
                    nc.vector.tensor_copy(out=xt_bf[:, ko], in_=xs)

            if use_bias:
                ones_bf = const.tile([1, P], bf16)
                nc.any.memset(ones_bf, 1.0)
                bias_sb = const.tile([1, N], fp32)
                nc.sync.dma_start(out=bias_sb, in_=bias[:])
                bias_bf = const.tile([1, N], bf16)
                nc.vector.tensor_copy(out=bias_bf, in_=bias_sb)

            for nt in range(NT):
                # W n-slice: stream fp32 in, tanh(kk*.) -> bf16 on ScalarE.
                wb = wpool.tile([P, KO, NTILE], bf16, tag="wb")
                for ko in range(KO):
                    ws = wstage.tile([P, NTILE], fp32, tag="ws")
                    nc.sync.dma_start(
                        out=ws, in_=w_r[:, ko, nt * NTILE:(nt + 1) * NTILE]
                    )
                    nc.scalar.activation(
                        out=wb[:, ko], in_=ws, func=wfunc, scale=wscale
                    )
                # k-outer accumulation into MT parallel PSUM banks: the PE
                # consumes x / W chunks in DMA arrival order, so it pipelines
                # with the loads instead of stalling on the full x tensor
                # inside the first accumulation group.
                ps = [
                    psum.tile([P, NTILE], fp32, tag="ps", name=f"ps{mt}")
                    for mt in range(MT)
                ]
                for ko in range(KO):
                    for mt in range(MT):
                        nc.tensor.matmul(
                            ps[mt],
                            xt_bf[:, ko, mt * P:(mt + 1) * P],
                            wb[:, ko],
                            start=(ko == 0),
                            stop=(ko == KO - 1) and not use_bias,
                        )
                for mt in range(MT):
                    if use_bias:
                        nc.tensor.matmul(
                            ps[mt],
                            ones_bf,
                            bias_bf[:, nt * NTILE:(nt + 1) * NTILE],
                            start=False,
                            stop=True,
                        )
                    ob = opool.tile([P, NTILE], fp32, tag="ob")
                    nc.vector.tensor_copy(out=ob, in_=ps[mt])
                    nc.sync.dma_start(
                        out=out_r[:, mt, nt * NTILE:(nt + 1) * NTILE], in_=ob
                    )

    nc.finalize()
    return nc


def kernel(x, kernel, bias, nmk, kk):
    from concourse.bass_utils import run_bass_kernel_spmd

    x = np.asarray(x)
    w = np.ascontiguousarray(np.asarray(kernel), dtype=np.float32)
    bias = np.asarray(bias, dtype=np.float32)
    nmk_f = float(np.asarray(nmk))
    kk_f = float(np.asarray(kk))

    M_full, K = x.shape
    _, N = w.shape
    assert M_full % N_CORES == 0
    M = M_full // N_CORES

    use_bias = bool(np.any(bias))

    key = (M, K, N, nmk_f, kk_f, use_bias)
    nc = _PROGRAM_CACHE.get(key)
    if nc is None:
        nc = _build_program(M, K, N, nmk_f, kk_f, use_bias)
        _PROGRAM_CACHE[key] = nc

    in_maps = []
    for i in range(N_CORES):
        m = {
            "xt": np.ascontiguousarray(x[i * M:(i + 1) * M, :].T, dtype=np.float32),
            "w": w,
        }
        if use_bias:
            m["bias"] = np.ascontiguousarray(bias.reshape(1, N))
        in_maps.append(m)

    # First 8-core execution of a freshly compiled NEFF is occasionally
    # flaky (NRT_EXEC_UNIT_UNRECOVERABLE); a retry reliably succeeds.
    last_exc = None
    for _attempt in range(3):
        try:
            res = run_bass_kernel_spmd(nc, in_maps, core_ids=list(range(N_CORES)))
            break
        except Exception as e:  # noqa: BLE001
            last_exc = e
    else:
        raise last_exc
    out = np.concatenate([r["out"] for r in res.results], axis=0)
    return out.astype(np.float32, copy=False)


# revision 7
# speedup vs baseline: 1.0319x; 1.0268x over previous
"""Trainium2 Bass kernel for nn_BinaryDense: y = nmk * (x @ tanh(kk*W)) + bias
(soft branch, kk < 1000) or y = nmk * (x @ sign(W)) + bias (hard branch).

Strategy: data-parallel shard of x over its row dim across 8 NeuronCores,
kernel/bias replicated. Per core: [1024, 4096] @ [4096, 4096] in bf16 with
fp32 PSUM accumulation. W is streamed through the scalar engine's tanh LUT
(fp32 -> bf16 cast fused); x is cast (and scaled by nmk) on the vector
engine once and stays SBUF-resident.
"""
import sys

sys.path.insert(0, "/opt/trn_rl_repo")

import numpy as np

N_CORES = 8
P = 128

KK_THRESHOLD = 1000.0

_PROGRAM_CACHE = {}


def _build_program(M, K, N, nmk, kk, use_bias):
    import concourse.bacc as bacc
    import concourse.mybir as mybir
    from concourse.tile import TileContext

    fp32 = mybir.dt.float32
    bf16 = mybir.dt.bfloat16

    KO = K // P          # k-tiles of 128
    MT = M // P          # out-tile rows
    NTILE = 512
    NT = N // NTILE      # out-tile cols

    nc = bacc.Bacc()
    xt = nc.dram_tensor("xt", [K, M], fp32, kind="ExternalInput")
    w = nc.dram_tensor("w", [K, N], fp32, kind="ExternalInput")
    if use_bias:
        bias = nc.dram_tensor("bias", [1, N], fp32, kind="ExternalInput")
    out = nc.dram_tensor("out", [M, N], fp32, kind="ExternalOutput")

    xt_r = xt.rearrange("(ko p) m -> p ko m", p=P)
    w_r = w.rearrange("(ko p) n -> p ko n", p=P)
    out_r = out.rearrange("(mo p) n -> p mo n", p=P)

    wfunc = (
        mybir.ActivationFunctionType.Tanh
        if kk < KK_THRESHOLD
        else mybir.ActivationFunctionType.Sign
    )
    wscale = float(kk) if kk < KK_THRESHOLD else 1.0

    with TileContext(nc) as tc:
        with tc.tile_pool(name="const", bufs=1) as const, \
             tc.tile_pool(name="xstage", bufs=4) as xstage, \
             tc.tile_pool(name="wstage", bufs=6) as wstage, \
             tc.tile_pool(name="wpool", bufs=2) as wpool, \
             tc.tile_pool(name="opool", bufs=4) as opool, \
             tc.tile_pool(name="psum", bufs=8, space="PSUM") as psum:

            # x: load fp32, scale by nmk, cast to bf16; stays resident.
            # Interleaved with the first W n-slice's loads so the PE can
            # start consuming (x[ko], W[ko]) chunk pairs in arrival order
            # instead of waiting behind the whole x transfer.
            xt_bf = const.tile([P, KO, M], bf16)
            wb0 = wpool.tile([P, KO, NTILE], bf16, tag="wb", name="wb0")
            for ko in range(KO):
                ws = wstage.tile([P, NTILE], fp32, tag="ws")
                nc.sync.dma_start(out=ws, in_=w_r[:, ko, 0:NTILE])
                nc.scalar.activation(
                    out=wb0[:, ko], in_=ws, func=wfunc, scale=wscale
                )
                xs = xstage.tile([P, M], fp32, tag="xs")
                nc.sync.dma_start(out=xs, in_=xt_r[:, ko])
                if nmk != 1.0:
                    nc.vector.tensor_scalar_mul(xt_bf[:, ko], xs, float(nmk))
                else:
                    nc.vector.tensor_copy(out=xt_bf[:, ko], in_=xs)

            if use_bias:
                ones_bf = const.tile([1, P], bf16)
                nc.any.memset(ones_bf, 1.0)
                bias_sb = const.tile([1, N], fp32)
                nc.sync.dma_start(out=bias_sb, in_=bias[:])
                bias_bf = const.tile([1, N], bf16)
                nc.vector.tensor_copy(out=bias_bf, in_=bias_sb)

            for nt in range(NT):
                if nt == 0:
                    wb = wb0
                else:
                    # W n-slice: stream fp32 in, tanh(kk*.) -> bf16 on ScalarE.
                    wb = wpool.tile([P, KO, NTILE], bf16, tag="wb", name="wb")
                    for ko in range(KO):
                        ws = wstage.tile([P, NTILE], fp32, tag="ws")
                        nc.sync.dma_start(
                            out=ws, in_=w_r[:, ko, nt * NTILE:(nt + 1) * NTILE]
                        )
                        nc.scalar.activation(
                            out=wb[:, ko], in_=ws, func=wfunc, scale=wscale
                        )
                # k-outer accumulation into MT parallel PSUM banks: the PE
                # consumes x / W chunks in DMA arrival order, so it pipelines
                # with the loads instead of stalling on the full x tensor
                # inside the first accumulation group.
                ps = [
                    psum.tile([P, NTILE], fp32, tag="ps", name=f"ps{mt}")
                    for mt in range(MT)
                ]
                for ko in range(KO):
                    for mt in range(MT):
                        nc.tensor.matmul(
                            ps[mt],
                            xt_bf[:, ko, mt * P:(mt + 1) * P],
                            wb[:, ko],
                            start=(ko == 0),
                            stop=(ko == KO - 1) and not use_bias,
                        )
                for mt in range(MT):
                    if use_bias:
                        nc.tensor.matmul(
                            ps[mt],
                            ones_bf,
                            bias_bf[:, nt * NTILE:(nt + 1) * NTILE],
                            start=False,
                            stop=True,
                        )
                    ob = opool.tile([P, NTILE], fp32, tag="ob")
                    nc.vector.tensor_copy(out=ob, in_=ps[mt])
                    nc.sync.dma_start(
                        out=out_r[:, mt, nt * NTILE:(nt + 1) * NTILE], in_=ob
                    )

    nc.finalize()
    return nc


def kernel(x, kernel, bias, nmk, kk):
    from concourse.bass_utils import run_bass_kernel_spmd

    x = np.asarray(x)
    w = np.ascontiguousarray(np.asarray(kernel), dtype=np.float32)
    bias = np.asarray(bias, dtype=np.float32)
    nmk_f = float(np.asarray(nmk))
    kk_f = float(np.asarray(kk))

    M_full, K = x.shape
    _, N = w.shape
    assert M_full % N_CORES == 0
    M = M_full // N_CORES

    use_bias = bool(np.any(bias))

    key = (M, K, N, nmk_f, kk_f, use_bias)
    nc = _PROGRAM_CACHE.get(key)
    if nc is None:
        nc = _build_program(M, K, N, nmk_f, kk_f, use_bias)
        _PROGRAM_CACHE[key] = nc

    in_maps = []
    for i in range(N_CORES):
        m = {
            "xt": np.ascontiguousarray(x[i * M:(i + 1) * M, :].T, dtype=np.float32),
            "w": w,
        }
        if use_bias:
            m["bias"] = np.ascontiguousarray(bias.reshape(1, N))
        in_maps.append(m)

    # First 8-core execution of a freshly compiled NEFF is occasionally
    # flaky (NRT_EXEC_UNIT_UNRECOVERABLE); a retry reliably succeeds.
    last_exc = None
    for _attempt in range(3):
        try:
            res = run_bass_kernel_spmd(nc, in_maps, core_ids=list(range(N_CORES)))
            break
        except Exception as e:  # noqa: BLE001
            last_exc = e
    else:
        raise last_exc
    out = np.concatenate([r["out"] for r in res.results], axis=0)
    return out.astype(np.float32, copy=False)


# revision 9
# speedup vs baseline: 1.0403x; 1.0082x over previous
"""Trainium2 Bass kernel for nn_BinaryDense: y = nmk * (x @ tanh(kk*W)) + bias
(soft branch, kk < 1000) or y = nmk * (x @ sign(W)) + bias (hard branch).

Strategy: data-parallel shard of x over its row dim across 8 NeuronCores,
kernel/bias replicated. Per core: [1024, 4096] @ [4096, 4096] in bf16 with
fp32 PSUM accumulation. W is streamed through the scalar engine's tanh LUT
(fp32 -> bf16 cast fused); x is cast (and scaled by nmk) on the vector
engine once and stays SBUF-resident.
"""
import sys

sys.path.insert(0, "/opt/trn_rl_repo")

import numpy as np

N_CORES = 8
P = 128

KK_THRESHOLD = 1000.0

_PROGRAM_CACHE = {}


def _build_program(M, K, N, nmk, kk, use_bias):
    import concourse.bacc as bacc
    import concourse.mybir as mybir
    from concourse.tile import TileContext

    fp32 = mybir.dt.float32
    bf16 = mybir.dt.bfloat16

    KO = K // P          # k-tiles of 128
    MT = M // P          # out-tile rows
    NTILE = 512
    NT = N // NTILE      # out-tile cols

    nc = bacc.Bacc()
    xt = nc.dram_tensor("xt", [K, M], fp32, kind="ExternalInput")
    w = nc.dram_tensor("w", [K, N], fp32, kind="ExternalInput")
    if use_bias:
        bias = nc.dram_tensor("bias", [1, N], fp32, kind="ExternalInput")
    # Output is produced transposed ([N, M]); the host un-transposes.
    out = nc.dram_tensor("out", [N, M], fp32, kind="ExternalOutput")

    xt_r = xt.rearrange("(ko p) m -> p ko m", p=P)
    w_r = w.rearrange("(ko p) n -> p ko n", p=P)
    out_r = out.rearrange("(no p) m -> p no m", p=P)

    wfunc = (
        mybir.ActivationFunctionType.Tanh
        if kk < KK_THRESHOLD
        else mybir.ActivationFunctionType.Sign
    )
    wscale = float(kk) if kk < KK_THRESHOLD else 1.0

    with TileContext(nc) as tc:
        with tc.tile_pool(name="const", bufs=1) as const, \
             tc.tile_pool(name="xstage", bufs=4) as xstage, \
             tc.tile_pool(name="wstage", bufs=6) as wstage, \
             tc.tile_pool(name="wpool", bufs=2) as wpool, \
             tc.tile_pool(name="opool", bufs=4) as opool, \
             tc.tile_pool(name="psum", bufs=8, space="PSUM") as psum:

            # x: load fp32, scale by nmk, cast to bf16; stays resident.
            # Interleaved with the first W n-slice's loads so the PE can
            # start consuming (x[ko], W[ko]) chunk pairs in arrival order
            # instead of waiting behind the whole x transfer.
            xt_bf = const.tile([P, KO, M], bf16)
            wb0 = wpool.tile([P, KO, NTILE], bf16, tag="wb", name="wb0")
            for ko in range(KO):
                ws = wstage.tile([P, NTILE], fp32, tag="ws")
                nc.sync.dma_start(out=ws, in_=w_r[:, ko, 0:NTILE])
                nc.scalar.activation(
                    out=wb0[:, ko], in_=ws, func=wfunc, scale=wscale
                )
                xs = xstage.tile([P, M], fp32, tag="xs")
                nc.sync.dma_start(out=xs, in_=xt_r[:, ko])
                if nmk != 1.0:
                    nc.vector.tensor_scalar_mul(xt_bf[:, ko], xs, float(nmk))
                else:
                    nc.vector.tensor_copy(out=xt_bf[:, ko], in_=xs)

            if use_bias:
                ones_bf = const.tile([1, NTILE], bf16)
                nc.any.memset(ones_bf, 1.0)
                bias_sb = const.tile([1, N], fp32)
                nc.sync.dma_start(out=bias_sb, in_=bias[:])
                bias_bf = const.tile([1, N], bf16)
                nc.vector.tensor_copy(out=bias_bf, in_=bias_sb)

            # MH: moving x chunks of 512 columns (M=1024 -> 2); NJ: 128-wide
            # W column tiles per n-group. NJ * MH PSUM banks per group.
            MH = M // NTILE
            NJ = 8 // MH
            NGROUPS = N // (NJ * P)
            for ng in range(NT):
                if ng == 0:
                    wb = wb0
                else:
                    # W n-slice: stream fp32 in, tanh(kk*.) -> bf16 on ScalarE.
                    wb = wpool.tile([P, KO, NTILE], bf16, tag="wb", name="wb")
                    for ko in range(KO):
                        ws = wstage.tile([P, NTILE], fp32, tag="ws")
                        nc.sync.dma_start(
                            out=ws, in_=w_r[:, ko, ng * NTILE:(ng + 1) * NTILE]
                        )
                        nc.scalar.activation(
                            out=wb[:, ko], in_=ws, func=wfunc, scale=wscale
                        )
                # k-outer accumulation into 8 parallel PSUM banks: the PE
                # consumes x / W chunks in DMA arrival order, so it pipelines
                # with the loads. W is the stationary operand (output is
                # transposed): each loaded W tile serves MH moving x chunks.
                ps = [
                    [
                        psum.tile([P, NTILE], fp32, tag="ps", name=f"ps{j}_{h}")
                        for h in range(MH)
                    ]
                    for j in range(NJ)
                ]
                for ko in range(KO):
                    for j in range(NJ):
                        for h in range(MH):
                            nc.tensor.matmul(
                                ps[j][h],
                                wb[:, ko, j * P:(j + 1) * P],
                                xt_bf[:, ko, h * NTILE:(h + 1) * NTILE],
                                start=(ko == 0),
                                stop=(ko == KO - 1) and not use_bias,
                            )
                for j in range(NJ):
                    for h in range(MH):
                        if use_bias:
                            nc.tensor.matmul(
                                ps[j][h],
                                bias_bf[:, ng * NTILE + j * P:ng * NTILE + (j + 1) * P],
                                ones_bf,
                                start=False,
                                stop=True,
                            )
                        ob = opool.tile([P, NTILE], fp32, tag="ob")
                        nc.vector.tensor_copy(out=ob, in_=ps[j][h])
                        nc.sync.dma_start(
                            out=out_r[:, ng * NJ + j, h * NTILE:(h + 1) * NTILE],
                            in_=ob,
                        )

    nc.finalize()
    return nc


def kernel(x, kernel, bias, nmk, kk):
    from concourse.bass_utils import run_bass_kernel_spmd

    x = np.asarray(x)
    w = np.ascontiguousarray(np.asarray(kernel), dtype=np.float32)
    bias = np.asarray(bias, dtype=np.float32)
    nmk_f = float(np.asarray(nmk))
    kk_f = float(np.asarray(kk))

    M_full, K = x.shape
    _, N = w.shape
    assert M_full % N_CORES == 0
    M = M_full // N_CORES

    use_bias = bool(np.any(bias))

    key = (M, K, N, nmk_f, kk_f, use_bias)
    nc = _PROGRAM_CACHE.get(key)
    if nc is None:
        nc = _build_program(M, K, N, nmk_f, kk_f, use_bias)
        _PROGRAM_CACHE[key] = nc

    in_maps = []
    for i in range(N_CORES):
        m = {
            "xt": np.ascontiguousarray(x[i * M:(i + 1) * M, :].T, dtype=np.float32),
            "w": w,
        }
        if use_bias:
            m["bias"] = np.ascontiguousarray(bias.reshape(1, N))
        in_maps.append(m)

    # First 8-core execution of a freshly compiled NEFF is occasionally
    # flaky (NRT_EXEC_UNIT_UNRECOVERABLE); a retry reliably succeeds.
    last_exc = None
    for _attempt in range(3):
        try:
            res = run_bass_kernel_spmd(nc, in_maps, core_ids=list(range(N_CORES)))
            break
        except Exception as e:  # noqa: BLE001
            last_exc = e
    else:
        raise last_exc
    out = np.concatenate([r["out"].T for r in res.results], axis=0)
    return out.astype(np.float32, copy=False)
